# revision 1
# baseline (speedup 1.0000x reference)
"""Trainium2 Bass kernel for 16-head MultiHeadAttention (B=4, S=2048, H=1024).

Sharding: 8 cores = (batch b in 0..3) x (head-group g in 0..1).
Core (b, g) computes batch b, heads 8g..8g+7 (512 of the 1024 projected dims)
and produces a partial output out_partial.T [1024, 2048] (f32). Host sums the
two partials per batch and transposes back.

On-core layout is fully "transposed" (seq on the free dim everywhere):
  XT  [H=1024, S]   = x[b].T                      (bf16, host-prepped)
  QT  [D=512,  S]   = Wq_g @ x.T + bq_g           (bf16)
  KT  [D=512,  S]                                  (bf16)
  V   [S, 8*(64+1)] = x @ Wv_g.T, plus a ones column per head (bf16)
  ST  [k, q] per (head, k-tile): scores^T          (psum f32)
  expST = exp(ST/8)                                (bf16)
  PV: psum[0:64] = V_h^T @ expST accumulated over k-tiles -> attnT (unnormalized)
      psum[64]   = ones @ expST = softmax denominator (ones-column trick)
  attnT [d, q] normalized via DVE reciprocal + DMA partition-broadcast
  outT  [o, q] = WoT_g^T @ attnT + bo_eff          (f32)

bv is folded away algebraically (softmax weights sum to 1):
  out = attn@V' + bv@Wo_g.T  -> host adds Wo@bv into bo_eff on the g=0 core.
"""

import os
from contextlib import ExitStack

import numpy as np
import ml_dtypes

import concourse.bass as bass
import concourse.mybir as mybir
import concourse.tile as tile
from concourse import bacc

BF16 = mybir.dt.bfloat16
F32 = mybir.dt.float32
AF = mybir.ActivationFunctionType
ALU = mybir.AluOpType

P = 128
H = 1024          # model dim
NH = 16           # total heads
HD = 64           # head dim
G = 2             # tensor-parallel head groups
D = H // G        # 512 per-core projected dim
NHG = NH // G     # 8 heads per core
HT = H // P       # 8 h-tiles (contraction tiles for projections)
DT = D // P       # 4 d-tiles
VW = NHG * (HD + 1)  # 520 V width incl ones columns
MMW = 512         # matmul moving free dim


def emit(tc: tile.TileContext, S: int):
    """Emit the single-core SPMD program. S parameterized for fast sim tests."""
    nc = tc.nc
    ST_ = S // P                  # s-tiles == k-tiles
    QW = min(1024, S)             # psum tile width (q chunk)
    QH = S // QW                  # q chunks per head
    NQC = max(1, QW // MMW)       # matmuls per psum tile
    W = QW // NQC                 # matmul free width

    xT = nc.declare_dram_parameter("xT", [H, S], BF16, isOutput=False)
    wqT = nc.declare_dram_parameter("wqT", [H, D], BF16, isOutput=False)
    wkT = nc.declare_dram_parameter("wkT", [H, D], BF16, isOutput=False)
    wvT = nc.declare_dram_parameter("wvT", [H, D], BF16, isOutput=False)
    woT = nc.declare_dram_parameter("woT", [D, H], BF16, isOutput=False)
    bqT = nc.declare_dram_parameter("bqT", [D], F32, isOutput=False)
    bkT = nc.declare_dram_parameter("bkT", [D], F32, isOutput=False)
    boE = nc.declare_dram_parameter("boE", [H], F32, isOutput=False)
    outT = nc.declare_dram_parameter("outT", [H, S], F32, isOutput=True)

    with ExitStack() as ctx:
        const = ctx.enter_context(tc.tile_pool(name="const", bufs=1))
        ps = ctx.enter_context(tc.tile_pool(name="ps", bufs=4, space="PSUM"))
        expp = ctx.enter_context(tc.tile_pool(name="expp", bufs=6))
        misc = ctx.enter_context(tc.tile_pool(name="misc", bufs=2))
        outp = ctx.enter_context(tc.tile_pool(name="outp", bufs=2))
        dramp = ctx.enter_context(tc.tile_pool(name="dramp", bufs=2, space="DRAM"))

        # ---- persistent SBUF tensors ----
        xt = const.tile([P, HT, S], BF16, tag="xt")
        wq = const.tile([P, HT, D], BF16, tag="wq")
        wk = const.tile([P, HT, D], BF16, tag="wk")
        wv = const.tile([P, HT, D], BF16, tag="wv")
        wo = const.tile([P, DT, H], BF16, tag="wo")
        bqv = const.tile([P, DT], F32, tag="bqv")
        bkv = const.tile([P, DT], F32, tag="bkv")
        bov = const.tile([P, HT], F32, tag="bov")
        qt = const.tile([P, DT, S], BF16, tag="qt")
        kt = const.tile([P, DT, S], BF16, tag="kt")
        vsb = const.tile([P, ST_, VW], BF16, tag="vsb")
        att = const.tile([P, DT, S], BF16, tag="att")

        # ---- input DMAs ----
        nc.sync.dma_start(xt[:], xT[:].rearrange("(ht p) s -> p ht s", p=P))
        nc.sync.dma_start(wq[:], wqT[:].rearrange("(ht p) d -> p ht d", p=P))
        nc.sync.dma_start(wk[:], wkT[:].rearrange("(ht p) d -> p ht d", p=P))
        nc.sync.dma_start(wv[:], wvT[:].rearrange("(ht p) d -> p ht d", p=P))
        nc.sync.dma_start(bqv[:], bqT[:].rearrange("(t p) -> p t", p=P))
        nc.sync.dma_start(bkv[:], bkT[:].rearrange("(t p) -> p t", p=P))
        nc.sync.dma_start(wo[:], woT[:].rearrange("(dt p) o -> p dt o", p=P))
        nc.sync.dma_start(bov[:], boE[:].rearrange("(t p) -> p t", p=P))

        # ones columns of V (column HD within each head's 65-wide stripe)
        v4 = vsb[:].rearrange("p st (h c) -> p st h c", c=HD + 1)
        nc.vector.memset(v4[:, :, :, HD : HD + 1], 1.0)

        # ---- QKV projection groups ----
        def proj_qk(dst, w, bias, dt, sh):
            """QT/KT d-tile: psum[p=d, q] = sum_ht w[ht,d-tile].T @ xt[ht, chunk]."""
            pt = ps.tile([P, QW], F32, tag="ps")
            for ht in range(HT):
                for qc in range(NQC):
                    nc.tensor.matmul(
                        pt[:, qc * W : (qc + 1) * W],
                        lhsT=w[:, ht, dt * P : (dt + 1) * P],
                        rhs=xt[:, ht, sh * QW + qc * W : sh * QW + (qc + 1) * W],
                        start=(ht == 0),
                        stop=(ht == HT - 1),
                    )
            nc.vector.tensor_scalar_add(
                dst[:, dt, sh * QW : (sh + 1) * QW], pt[:], bias[:, dt : dt + 1]
            )

        def proj_v(sv, nst):
            """V group: nst s-tiles, natural [s, d] layout, scattered into the
            65-wide per-head stripes of vsb."""
            pt = ps.tile([P, QW], F32, tag="ps")
            for stl in range(nst):
                stile = sv * nst + stl
                for ht in range(HT):
                    nc.tensor.matmul(
                        pt[:, stl * D : (stl + 1) * D],
                        lhsT=xt[:, ht, stile * P : (stile + 1) * P],
                        rhs=wv[:, ht, :],
                        start=(ht == 0),
                        stop=(ht == HT - 1),
                    )
            src = pt[:, : nst * D].rearrange("p (s h c) -> p s h c", s=nst, h=NHG, c=HD)
            dst = v4[:, sv * nst : (sv + 1) * nst, :, 0:HD]
            nc.vector.tensor_copy(dst, src)

        # ---- attention for one head, one q-chunk ----
        def head_qchunk(h, qh):
            dt, off = h // 2, (h % 2) * HD
            pv = ps.tile([P, QW], F32, tag="ps")
            for kt_i in range(ST_):
                st = ps.tile([P, QW], F32, tag="ps")
                for qc in range(NQC):
                    nc.tensor.matmul(
                        st[:, qc * W : (qc + 1) * W],
                        lhsT=kt[off : off + HD, dt, kt_i * P : (kt_i + 1) * P],
                        rhs=qt[off : off + HD, dt, qh * QW + qc * W : qh * QW + (qc + 1) * W],
                        start=True,
                        stop=True,
                    )
                ex = expp.tile([P, QW], BF16, tag="ex")
                nc.scalar.activation(ex[:], st[:], AF.Exp, scale=0.125)
                for qc in range(NQC):
                    nc.tensor.matmul(
                        pv[0 : HD + 1, qc * W : (qc + 1) * W],
                        lhsT=vsb[:, kt_i, h * (HD + 1) : (h + 1) * (HD + 1)],
                        rhs=ex[:, qc * W : (qc + 1) * W],
                        start=(kt_i == 0),
                        stop=(kt_i == ST_ - 1),
                    )
            # softmax denominators -> reciprocal -> partition-broadcast via DRAM
            rec = misc.tile([1, QW], F32, tag="rec")
            nc.vector.reciprocal(rec[:], pv[HD : HD + 1, :])
            dsc = dramp.tile([1, QW], F32, tag="dsc")
            nc.sync.dma_start(dsc[:], rec[:])
            rrep = misc.tile([HD, QW], F32, tag="rrep")
            nc.sync.dma_start(rrep[:], dsc[0:1, :].to_broadcast((HD, QW)))
            nc.vector.tensor_tensor(
                att[off : off + HD, dt, qh * QW : (qh + 1) * QW],
                pv[0:HD, :],
                rrep[:],
                ALU.mult,
            )

        # ---- emission order: interleave projections with attention so ACT's
        # exp stream overlaps projection matmuls ----
        for sh in range(QH):
            proj_qk(qt, wq, bqv, 0, sh)
        for sh in range(QH):
            proj_qk(kt, wk, bkv, 0, sh)
        nvg = max(1, ST_ // (QW // D))     # V groups cover all s-tiles
        nst = ST_ // nvg
        for sv in range(nvg):
            proj_v(sv, nst)
        for dt in range(1, DT):
            for hh in range(2):
                h = (dt - 1) * 2 + hh
                for qh in range(QH):
                    head_qchunk(h, qh)
            for sh in range(QH):
                proj_qk(qt, wq, bqv, dt, sh)
            for sh in range(QH):
                proj_qk(kt, wk, bkv, dt, sh)
        for h in range(2 * (DT - 1), NHG):
            for qh in range(QH):
                head_qchunk(h, qh)

        # ---- output projection ----
        ot_view = outT[:].rearrange("(ot p) s -> p ot s", p=P)
        for ot in range(HT):
            for qh in range(QH):
                pt = ps.tile([P, QW], F32, tag="ps")
                for dt in range(DT):
                    for qc in range(NQC):
                        nc.tensor.matmul(
                            pt[:, qc * W : (qc + 1) * W],
                            lhsT=wo[:, dt, ot * P : (ot + 1) * P],
                            rhs=att[:, dt, qh * QW + qc * W : qh * QW + (qc + 1) * W],
                            start=(dt == 0),
                            stop=(dt == DT - 1),
                        )
                ob = outp.tile([P, QW], F32, tag="ob")
                nc.vector.tensor_scalar_add(ob[:], pt[:], bov[:, ot : ot + 1])
                nc.sync.dma_start(ot_view[:, ot, qh * QW : (qh + 1) * QW], ob[:])


def build_module(S: int = 2048):
    nc = bacc.Bacc("TRN2", target_bir_lowering=False, debug=False)
    with tile.TileContext(nc) as tc:
        emit(tc, S)
    nc.compile()
    return nc


def make_in_maps(x, Wq, bq, Wk, bk, Wv, bv, Wo, bo):
    """Host-side shard + layout prep. Core c = 2*b + g."""
    bf16 = ml_dtypes.bfloat16
    bo_eff = (bo + Wo.astype(np.float64) @ bv.astype(np.float64)).astype(np.float32)
    in_maps = []
    for b in range(4):
        xTb = np.ascontiguousarray(x[b].T).astype(bf16)
        for g in range(G):
            sl = slice(g * D, (g + 1) * D)
            in_maps.append(
                {
                    "xT": xTb,
                    "wqT": np.ascontiguousarray(Wq[sl, :].T).astype(bf16),
                    "wkT": np.ascontiguousarray(Wk[sl, :].T).astype(bf16),
                    "wvT": np.ascontiguousarray(Wv[sl, :].T).astype(bf16),
                    "woT": np.ascontiguousarray(Wo[:, sl].T).astype(bf16),
                    "bqT": np.ascontiguousarray(bq[sl]).astype(np.float32),
                    "bkT": np.ascontiguousarray(bk[sl]).astype(np.float32),
                    "boE": bo_eff if g == 0 else np.zeros(H, np.float32),
                }
            )
    return in_maps


_NC_CACHE = {}


def _get_module(S=2048):
    if S not in _NC_CACHE:
        _NC_CACHE[S] = build_module(S)
    return _NC_CACHE[S]


def kernel(x, Wq, bq, Wk, bk, Wv, bv, Wo, bo):
    from concourse.bass_utils import run_bass_kernel_spmd

    nc = _get_module(x.shape[1])
    in_maps = make_in_maps(x, Wq, bq, Wk, bk, Wv, bv, Wo, bo)
    trace = bool(int(os.environ.get("KERNEL_TRACE", "0")))
    res = run_bass_kernel_spmd(nc, in_maps, core_ids=list(range(8)), trace=trace)
    kernel.last_results = res
    out = np.empty((4, x.shape[1], H), np.float32)
    for b in range(4):
        acc = res.results[2 * b]["outT"] + res.results[2 * b + 1]["outT"]
        out[b] = acc.T
    return out


# revision 4
# speedup vs baseline: 1.4176x; 1.4176x over previous
"""Trainium2 Bass kernel for 16-head MultiHeadAttention (B=4, S=2048, H=1024).

Sharding: 8 cores = (batch b in 0..3) x (head-group g in 0..1).
Core (b, g) computes batch b, heads 8g..8g+7 (512 of the 1024 projected dims)
and produces a partial output out_partial.T [1024, 2048] (f32). Host sums the
two partials per batch and transposes back.

On-core layout is fully "transposed" (seq on the free dim everywhere):
  XT  [H=1024, S]   = x[b].T                      (bf16, host-prepped)
  QT  [D=512,  S]   = Wq_g @ x.T + bq_g           (bf16)
  KT  [D=512,  S]                                  (bf16)
  V   [S, 8*(64+1)] = x @ Wv_g.T, plus a ones column per head (bf16)
  ST  [k, q] per (head, k-tile): scores^T          (psum f32)
  expST = exp(ST/8)                                (bf16)
  PV: psum[0:64] = V_h^T @ expST accumulated over k-tiles -> attnT (unnormalized)
      psum[64]   = ones @ expST = softmax denominator (ones-column trick)
  attnT [d, q] normalized via DVE reciprocal + DMA partition-broadcast
  outT  [o, q] = WoT_g^T @ attnT + bo_eff          (f32)

bv is folded away algebraically (softmax weights sum to 1):
  out = attn@V' + bv@Wo_g.T  -> host adds Wo@bv into bo_eff on the g=0 core.
"""

import os
from contextlib import ExitStack

import numpy as np
import ml_dtypes

import concourse.bass as bass
import concourse.mybir as mybir
import concourse.tile as tile
from concourse import bacc

BF16 = mybir.dt.bfloat16
F32 = mybir.dt.float32
AF = mybir.ActivationFunctionType
ALU = mybir.AluOpType

P = 128
H = 1024          # model dim
NH = 16           # total heads
HD = 64           # head dim
G = 2             # tensor-parallel head groups
D = H // G        # 512 per-core projected dim
NHG = NH // G     # 8 heads per core
HT = H // P       # 8 h-tiles (contraction tiles for projections)
DT = D // P       # 4 d-tiles
VW = NHG * (HD + 1)  # 520 V width incl ones columns
MMW = 512         # matmul moving free dim


def emit(tc: tile.TileContext, S: int):
    """Emit the single-core SPMD program. S parameterized for fast sim tests."""
    nc = tc.nc
    ST_ = S // P                  # s-tiles == k-tiles
    QW = min(1024, S)             # psum tile width (q chunk)
    QH = S // QW                  # q chunks per head
    NQC = max(1, QW // MMW)       # matmuls per psum tile
    W = QW // NQC                 # matmul free width

    xT = nc.declare_dram_parameter("xT", [H, S], BF16, isOutput=False)
    wqT = nc.declare_dram_parameter("wqT", [H, D], BF16, isOutput=False)
    wkT = nc.declare_dram_parameter("wkT", [H, D], BF16, isOutput=False)
    wvT = nc.declare_dram_parameter("wvT", [H, D], BF16, isOutput=False)
    woT = nc.declare_dram_parameter("woT", [D, H], BF16, isOutput=False)
    bqT = nc.declare_dram_parameter("bqT", [D], F32, isOutput=False)
    bkT = nc.declare_dram_parameter("bkT", [D], F32, isOutput=False)
    boE = nc.declare_dram_parameter("boE", [H], F32, isOutput=False)
    outT = nc.declare_dram_parameter("outT", [H, S], F32, isOutput=True)

    with ExitStack() as ctx:
        const = ctx.enter_context(tc.tile_pool(name="const", bufs=1))
        ps = ctx.enter_context(tc.tile_pool(name="ps", bufs=4, space="PSUM"))
        expp = ctx.enter_context(tc.tile_pool(name="expp", bufs=6))
        misc = ctx.enter_context(tc.tile_pool(name="misc", bufs=2))
        outp = ctx.enter_context(tc.tile_pool(name="outp", bufs=2))
        dramp = ctx.enter_context(tc.tile_pool(name="dramp", bufs=2, space="DRAM"))

        # ---- persistent SBUF tensors ----
        xt = const.tile([P, HT, S], BF16, tag="xt")
        wq = const.tile([P, HT, D], BF16, tag="wq")
        wk = const.tile([P, HT, D], BF16, tag="wk")
        wv = const.tile([P, HT, D], BF16, tag="wv")
        wo = const.tile([P, DT, H], BF16, tag="wo")
        bqv = const.tile([P, DT], F32, tag="bqv")
        bkv = const.tile([P, DT], F32, tag="bkv")
        bov = const.tile([P, HT], F32, tag="bov")
        qt = const.tile([P, DT, S], BF16, tag="qt")
        kt = const.tile([P, DT, S], BF16, tag="kt")
        vsb = const.tile([P, ST_, VW], BF16, tag="vsb")
        att = const.tile([P, DT, S], BF16, tag="att")

        # ---- input DMAs ----
        nc.sync.dma_start(xt[:], xT[:].rearrange("(ht p) s -> p ht s", p=P))
        nc.sync.dma_start(wq[:], wqT[:].rearrange("(ht p) d -> p ht d", p=P))
        nc.sync.dma_start(wk[:], wkT[:].rearrange("(ht p) d -> p ht d", p=P))
        nc.sync.dma_start(wv[:], wvT[:].rearrange("(ht p) d -> p ht d", p=P))
        nc.sync.dma_start(bqv[:], bqT[:].rearrange("(t p) -> p t", p=P))
        nc.sync.dma_start(bkv[:], bkT[:].rearrange("(t p) -> p t", p=P))
        nc.sync.dma_start(wo[:], woT[:].rearrange("(dt p) o -> p dt o", p=P))
        nc.sync.dma_start(bov[:], boE[:].rearrange("(t p) -> p t", p=P))

        # ones columns of V (column HD within each head's 65-wide stripe)
        v4 = vsb[:].rearrange("p st (h c) -> p st h c", c=HD + 1)
        nc.vector.memset(v4[:, :, :, HD : HD + 1], 1.0)

        # ---- QKV projection groups ----
        def proj_qk(dst, w, bias, dt, sh):
            """QT/KT d-tile: psum[p=d, q] = sum_ht w[ht,d-tile].T @ xt[ht, chunk]."""
            pt = ps.tile([P, QW], F32, tag="ps")
            for ht in range(HT):
                for qc in range(NQC):
                    nc.tensor.matmul(
                        pt[:, qc * W : (qc + 1) * W],
                        lhsT=w[:, ht, dt * P : (dt + 1) * P],
                        rhs=xt[:, ht, sh * QW + qc * W : sh * QW + (qc + 1) * W],
                        start=(ht == 0),
                        stop=(ht == HT - 1),
                    )
            nc.vector.tensor_scalar_add(
                dst[:, dt, sh * QW : (sh + 1) * QW], pt[:], bias[:, dt : dt + 1]
            )

        def proj_v(sv, nst):
            """V group: nst s-tiles, natural [s, d] layout, scattered into the
            65-wide per-head stripes of vsb."""
            pt = ps.tile([P, QW], F32, tag="ps")
            for stl in range(nst):
                stile = sv * nst + stl
                for ht in range(HT):
                    nc.tensor.matmul(
                        pt[:, stl * D : (stl + 1) * D],
                        lhsT=xt[:, ht, stile * P : (stile + 1) * P],
                        rhs=wv[:, ht, :],
                        start=(ht == 0),
                        stop=(ht == HT - 1),
                    )
            src = pt[:, : nst * D].rearrange("p (s h c) -> p s h c", s=nst, h=NHG, c=HD)
            dst = v4[:, sv * nst : (sv + 1) * nst, :, 0:HD]
            nc.vector.tensor_copy(dst, src)

        # ---- attention for one head: all q-chunks interleaved, PV lagged 2
        # k-tiles behind scores so the PE never stalls on ACT's exp ----
        LAG = 2

        def head(h):
            dt, off = h // 2, (h % 2) * HD
            pvs = [ps.tile([P, QW], F32, tag="ps", name=f"pv{qh}") for qh in range(QH)]
            exs = {}
            for step in range(ST_ + LAG):
                if step < ST_:
                    kt_i = step
                    for qh in range(QH):
                        st = ps.tile([P, QW], F32, tag="ps", name="st")
                        for qc in range(NQC):
                            nc.tensor.matmul(
                                st[:, qc * W : (qc + 1) * W],
                                lhsT=kt[off : off + HD, dt, kt_i * P : (kt_i + 1) * P],
                                rhs=qt[off : off + HD, dt, qh * QW + qc * W : qh * QW + (qc + 1) * W],
                                start=True,
                                stop=True,
                            )
                        ex = expp.tile([P, QW], BF16, tag="ex")
                        nc.scalar.activation(ex[:], st[:], AF.Exp, scale=0.125)
                        exs[(qh, kt_i)] = ex
                if step >= LAG:
                    kt_j = step - LAG
                    for qh in range(QH):
                        ex = exs.pop((qh, kt_j))
                        for qc in range(NQC):
                            nc.tensor.matmul(
                                pvs[qh][0 : HD + 1, qc * W : (qc + 1) * W],
                                lhsT=vsb[:, kt_j, h * (HD + 1) : (h + 1) * (HD + 1)],
                                rhs=ex[:, qc * W : (qc + 1) * W],
                                start=(kt_j == 0),
                                stop=(kt_j == ST_ - 1),
                            )
            for qh in range(QH):
                pv = pvs[qh]
                # evict unnormalized + denom row (frees PSUM fast), then
                # normalize out-of-line via DMA partition-broadcast + approx recip
                attu = misc.tile([HD + 1, QW], F32, tag="attu")
                nc.vector.tensor_copy(attu[:], pv[0 : HD + 1, :])
                dsc = dramp.tile([1, QW], F32, tag="dsc")
                nc.sync.dma_start(dsc[:], attu[HD : HD + 1, :])
                denr = misc.tile([HD, QW], F32, tag="denr")
                nc.sync.dma_start(denr[:], dsc[0:1, :].to_broadcast((HD, QW)))
                recr = misc.tile([HD, QW], F32, tag="recr")
                nc.vector.reciprocal_approx_fast(recr[:], denr[:])
                nc.vector.tensor_tensor(
                    att[off : off + HD, dt, qh * QW : (qh + 1) * QW],
                    attu[0:HD, :],
                    recr[:],
                    ALU.mult,
                )

        # ---- emission order: interleave projections with attention so ACT's
        # exp stream overlaps projection matmuls ----
        for sh in range(QH):
            proj_qk(qt, wq, bqv, 0, sh)
        for sh in range(QH):
            proj_qk(kt, wk, bkv, 0, sh)
        nvg = max(1, ST_ // (QW // D))     # V groups cover all s-tiles
        nst = ST_ // nvg
        for sv in range(nvg):
            proj_v(sv, nst)
        for dt in range(1, DT):
            for hh in range(2):
                head((dt - 1) * 2 + hh)
            for sh in range(QH):
                proj_qk(qt, wq, bqv, dt, sh)
            for sh in range(QH):
                proj_qk(kt, wk, bkv, dt, sh)
        for h in range(2 * (DT - 1), NHG):
            head(h)

        # ---- output projection ----
        ot_view = outT[:].rearrange("(ot p) s -> p ot s", p=P)
        for ot in range(HT):
            for qh in range(QH):
                pt = ps.tile([P, QW], F32, tag="ps")
                for dt in range(DT):
                    for qc in range(NQC):
                        nc.tensor.matmul(
                            pt[:, qc * W : (qc + 1) * W],
                            lhsT=wo[:, dt, ot * P : (ot + 1) * P],
                            rhs=att[:, dt, qh * QW + qc * W : qh * QW + (qc + 1) * W],
                            start=(dt == 0),
                            stop=(dt == DT - 1),
                        )
                ob = outp.tile([P, QW], F32, tag="ob")
                nc.vector.tensor_scalar_add(ob[:], pt[:], bov[:, ot : ot + 1])
                nc.sync.dma_start(ot_view[:, ot, qh * QW : (qh + 1) * QW], ob[:])


def build_module(S: int = 2048):
    nc = bacc.Bacc("TRN2", target_bir_lowering=False, debug=False)
    with tile.TileContext(nc) as tc:
        emit(tc, S)
    nc.compile()
    return nc


def make_in_maps(x, Wq, bq, Wk, bk, Wv, bv, Wo, bo):
    """Host-side shard + layout prep. Core c = 2*b + g."""
    bf16 = ml_dtypes.bfloat16
    bo_eff = (bo + Wo.astype(np.float64) @ bv.astype(np.float64)).astype(np.float32)
    in_maps = []
    for b in range(4):
        xTb = np.ascontiguousarray(x[b].T).astype(bf16)
        for g in range(G):
            sl = slice(g * D, (g + 1) * D)
            in_maps.append(
                {
                    "xT": xTb,
                    "wqT": np.ascontiguousarray(Wq[sl, :].T).astype(bf16),
                    "wkT": np.ascontiguousarray(Wk[sl, :].T).astype(bf16),
                    "wvT": np.ascontiguousarray(Wv[sl, :].T).astype(bf16),
                    "woT": np.ascontiguousarray(Wo[:, sl].T).astype(bf16),
                    "bqT": np.ascontiguousarray(bq[sl]).astype(np.float32),
                    "bkT": np.ascontiguousarray(bk[sl]).astype(np.float32),
                    "boE": bo_eff if g == 0 else np.zeros(H, np.float32),
                }
            )
    return in_maps


_NC_CACHE = {}


def _get_module(S=2048):
    if S not in _NC_CACHE:
        _NC_CACHE[S] = build_module(S)
    return _NC_CACHE[S]


def kernel(x, Wq, bq, Wk, bk, Wv, bv, Wo, bo):
    from concourse.bass_utils import run_bass_kernel_spmd

    nc = _get_module(x.shape[1])
    in_maps = make_in_maps(x, Wq, bq, Wk, bk, Wv, bv, Wo, bo)
    trace = bool(int(os.environ.get("KERNEL_TRACE", "0")))
    res = run_bass_kernel_spmd(nc, in_maps, core_ids=list(range(8)), trace=trace)
    kernel.last_results = res
    out = np.empty((4, x.shape[1], H), np.float32)
    for b in range(4):
        acc = res.results[2 * b]["outT"] + res.results[2 * b + 1]["outT"]
        out[b] = acc.T
    return out
